# revision 32
# baseline (speedup 1.0000x reference)
"""BitfieldLinear (vq_codebook) Trainium2 kernel — fp8 decomposed.

y = x @ W^T + bias, W = r[:,None]*basis[idx] + s[:,None]*(q-128)/127.

Instead of materializing W in bf16 (PE floor ~437us/core), split:
  y = zT.T @ G + x_f8 @ residT_f8 * (1/C) + bias
  - z = x @ basisT in bf16; the basis term becomes a one-hot gather
    matmul with G[b,o] = C*r[o] at b=idx[o] (accurate path, small).
  - the residual matmul runs in fp8e4 DoubleRow perf mode (2x PE rate);
    residT_f8 pre-scaled by s*C/127 (C=2048 keeps values out of the
    fp8 subnormal range), x cast bf16->fp8 directly.
  Both accumulate into the same PSUM bank; evacuation is one DVE op
  (psum * 1/C + bias_bc).

Sharding across 8 NeuronCores: 2-way over out_features x 4-way over
flattened tokens (2048 tokens x 2048 outs per core, K=4096). The
out-feature split is BUCKETED by basis index: the host partitions the
256 basis rows into two sets whose assigned output counts are exactly
2048 each (dense subset-sum on the actual codes), permutes the
o-axis inputs accordingly, and un-permutes y on gather. Each core
then needs only its 128 basis rows (halves the z/base-term work).
Falls back to full-256-basis replication if no exact split exists.

All transposes run on the PE (the DMA xbar is descriptor-rate-bound);
x streams via gpsimd cast-DMA (f32->bf16); resid rows stream as
full-row i32 loads (16KB descriptors) alternating sync/scalar HWDGE
queues; decode on ACT with s*C/127 folded; PSUM evacuations on DVE.
"""

import numpy as np

import concourse.bass as bass
import concourse.mybir as mybir
import concourse.tile as tile
from concourse.masks import make_identity
from concourse.bass_utils import run_bass_kernel_spmd

# problem shape (hardcoded per harness contract)
B, S, D_IN, D_OUT, BASIS = 4, 2048, 4096, 4096, 256
N_CORES = 8
O_SHARDS, N_SHARDS = 2, 4           # grid: core = oc * N_SHARDS + nb
O_SH = D_OUT // O_SHARDS            # 2048 out-features per core
N_SH = (B * S) // N_SHARDS          # 2048 token rows per core

P = 128
KC = D_IN // P                      # 32 contraction chunks
KH = KC // 2                        # 16 chunks per D_IN half
HALF = D_IN // 2
NB = N_SH // P                      # 16 token blocks per core
NOS = O_SH // 512                   # 4 PSUM o-slices per core
OT = O_SH // P                      # 16 resid row-tiles per core
CSC = 2048.0                        # fp8 residual pre-scale

F32 = mybir.dt.float32
BF16 = mybir.dt.bfloat16
F8 = mybir.dt.float8e4
I32 = mybir.dt.int32

_WAIT_LIMIT = 1


def _split_sync_waits(nc):
    """walrus in this container rejects instructions with more than one
    embedded sync-wait command; hoist the excess onto same-engine NoOps."""
    ctr = 0
    for f in nc.m.functions:
        for bb in f.blocks:
            new = []
            changed = False
            for inst in bb.instructions:
                si = inst.sync_info
                if si is not None and si.on_wait and len(si.on_wait) > _WAIT_LIMIT:
                    waits = list(si.on_wait)
                    excess, keep = waits[:-_WAIT_LIMIT], waits[-_WAIT_LIMIT:]
                    for i in range(0, len(excess), _WAIT_LIMIT):
                        ctr += 1
                        new.append(mybir.InstNoOp(
                            name=f"I-waitsplit-{ctr}",
                            engine=inst.engine,
                            ins=[], outs=[],
                            sync_info=mybir.SyncInfo(
                                on_wait=excess[i:i + _WAIT_LIMIT], on_update=[]),
                        ))
                    si.on_wait = keep
                    changed = True
                new.append(inst)
            if changed:
                bb.instructions = new


def _build_program(nbh, split_waits=True):
    """nbh = basis halves per core: 1 (bucketed, 128 rows) or 2 (full)."""
    nc = bass.Bass()
    Alu = mybir.AluOpType
    Act = mybir.ActivationFunctionType
    DR = mybir.MatmulPerfMode.DoubleRow
    BL = nbh * P                    # local basis rows

    x_in = nc.dram_tensor("x_sh", [N_SH, D_IN], F32, kind="ExternalInput")
    codes_in = nc.dram_tensor("codes_sh", [O_SH], I32, kind="ExternalInput")
    basis_in = nc.dram_tensor("basis", [BL, D_IN], F32, kind="ExternalInput")
    resid_in = nc.dram_tensor("resid_sh", [O_SH, D_IN], I32, kind="ExternalInput")
    scales_in = nc.dram_tensor("scales_sh", [O_SH], F32, kind="ExternalInput")
    bias_in = nc.dram_tensor("bias_sh", [O_SH], F32, kind="ExternalInput")
    y_out = nc.dram_tensor("y_sh", [N_SH, O_SH], F32, kind="ExternalOutput")

    with tile.TileContext(nc) as tc:
        with (
            tc.tile_pool(name="const", bufs=1) as cpool,
            tc.tile_pool(name="rows", bufs=2) as rowpool,   # [1,512] i32
            tc.tile_pool(name="xbf", bufs=2 if nbh == 1 else 1) as xbfpool,
            tc.tile_pool(name="xt", bufs=2) as xtpool,      # [128,KC,128] bf16
            tc.tile_pool(name="xf8", bufs=4) as xf8pool,    # [128,KC,128] f8
            tc.tile_pool(name="r32", bufs=2) as r32pool,    # [128,4096] i32
            tc.tile_pool(name="rbf", bufs=2 if nbh == 1 else 1) as rbfpool,    # [128,4096] bf16
            tc.tile_pool(name="y", bufs=2) as ypool,        # [128,512] f32
            tc.tile_pool(name="psum", bufs=6, space="PSUM") as pspool,
        ):
            # ---- decode code scalars (chunked to keep scratch small) -
            idx_row_f = cpool.tile([1, O_SH], BF16, name="idx_row_f")
            r_row_f = cpool.tile([1, O_SH], BF16, name="r_row_f")
            for cq in range(4):
                cs = slice(cq * 512, (cq + 1) * 512)
                c_row = rowpool.tile([1, 512], I32, tag="row",
                                     name=f"c_row{cq}")
                nc.sync.dma_start(c_row[:], codes_in[None, cs])
                t_row = rowpool.tile([1, 512], I32, tag="row",
                                     name=f"t_row{cq}")
                nc.vector.tensor_scalar(t_row[:], c_row[:], 0xFF, None,
                                        Alu.bitwise_and)
                nc.scalar.activation(idx_row_f[:, cs], t_row[:], Act.Copy)
                nc.vector.tensor_scalar(t_row[:], c_row[:], 8, 0xFFFF,
                                        Alu.logical_shift_right,
                                        Alu.bitwise_and)
                # r scaled by C so the fp8 residual 1/C evac matches
                nc.scalar.activation(r_row_f[:, cs], t_row[:], Act.Copy,
                                     scale=CSC / 65535.0)

            # per-row decode scale/bias for the residual (ACT layout);
            # [16,128] load + PE transpose (the AP-swap DMA is 2048
            # 4-byte descriptors and stalls the sync queue at startup)
            s16 = cpool.tile([OT, P], F32, name="s16")
            nc.sync.dma_start(s16[:], scales_in.rearrange("(t p) -> t p", p=P))
            id16 = cpool.tile([OT, OT], F32, name="id16")
            make_identity(nc, id16[:])
            ps_s = pspool.tile([P, 512], F32, tag="bc", bufs=3, name="ps_s")
            nc.tensor.transpose(ps_s[:, :OT], s16[:], id16[:])
            sv_pp = cpool.tile([P, OT], F32, name="sv_pp")
            nc.vector.tensor_scalar_mul(sv_pp[:], ps_s[:, :OT], CSC / 127.0)
            bv_pp = cpool.tile([P, OT], F32, name="bv_pp")
            nc.vector.tensor_scalar_mul(bv_pp[:], ps_s[:, :OT],
                                        -128.0 * CSC / 127.0)

            bias_row = cpool.tile([1, O_SH], BF16, name="bias_row")
            nc.gpsimd.dma_start(bias_row[:], bias_in[None, :])
            ones_row = cpool.tile([1, P], BF16, name="ones_row")
            nc.vector.memset(ones_row[:], 1.0)
            identity = cpool.tile([P, P], BF16, name="identity")
            make_identity(nc, identity[:])

            # ---- one-hot G [128 b, nbh, O_SH o] and bias broadcast ---
            iota_i = cpool.tile([P, 1], I32, name="iota_i")
            nc.gpsimd.iota(iota_i[:], pattern=[[0, 1]], base=0,
                           channel_multiplier=1)
            iota_f = [cpool.tile([P, 1], F32, name=f"iota_f{bh}")
                      for bh in range(nbh)]
            nc.scalar.activation(iota_f[0][:], iota_i[:], Act.Copy)
            if nbh == 2:
                nc.scalar.activation(iota_f[1][:], iota_i[:], Act.Copy,
                                     bias=128.0, scale=1.0)
            G = cpool.tile([P, nbh, O_SH], BF16, name="G")
            bias_bc = cpool.tile([P, O_SH], BF16, name="bias_bc")
            r_bc = cpool.tile([P, 512], BF16, name="r_bc")
            for q in range(NOS):
                qs = slice(q * 512, (q + 1) * 512)
                pr = pspool.tile([P, 512], F32, tag="bc", bufs=3, name=f"pr{q}")
                nc.tensor.matmul(pr[:], lhsT=ones_row[:], rhs=r_row_f[:, qs],
                                 start=True, stop=True)
                nc.scalar.copy(r_bc[:], pr[:])
                pi = pspool.tile([P, 512], F32, tag="bc", bufs=3, name=f"pi{q}")
                nc.tensor.matmul(pi[:], lhsT=ones_row[:], rhs=idx_row_f[:, qs],
                                 start=True, stop=True)
                for bh in range(nbh):
                    nc.vector.scalar_tensor_tensor(
                        G[:, bh, qs], pi[:], iota_f[bh][:, :1], r_bc[:],
                        op0=Alu.is_equal, op1=Alu.mult)
                pb = pspool.tile([P, 512], F32, tag="bc", bufs=3, name=f"pb{q}")
                nc.tensor.matmul(pb[:], lhsT=ones_row[:], rhs=bias_row[:, qs],
                                 start=True, stop=True)
                nc.scalar.copy(bias_bc[:, qs], pb[:])

            # ---- basisT [128 i, KC k, BL b] bf16 ---------------------
            basisT = cpool.tile([P, KC, BL], BF16, name="basisT")
            for bh2 in range(nbh):
                b_f32 = r32pool.tile([P, D_IN], F32, tag="r32",
                                     name=f"bf32_{bh2}")
                nc.scalar.dma_start(b_f32[:], basis_in[bh2 * P:(bh2 + 1) * P, :])
                b_bf = xbfpool.tile([P, D_IN], BF16, tag="xbf",
                                    name=f"bbf{bh2}")
                nc.vector.tensor_copy(b_bf[:], b_f32[:])
                nc.sync.dma_start_transpose(
                    basisT[:, :, bh2 * P:(bh2 + 1) * P], b_bf[:])

            # ---- persistent W^T (residual, fp8) and zT ---------------
            residT = cpool.tile([P, KC, O_SH], F8, name="residT")
            zT = cpool.tile([P, nbh, N_SH], BF16, name="zT")

            def resid_step(t):
                # [128 o-rows, 4096 i]: full-row load (16KB descriptors),
                # ACT decode, PE transpose, DVE evac casts to f8
                r32 = r32pool.tile([P, D_IN], I32, tag="r32", name=f"r32_{t}")
                eng_a = nc.sync if t % 2 == 0 else nc.scalar
                eng_a.dma_start(r32[:], resid_in[t * P:(t + 1) * P, :])
                rbf = rbfpool.tile([P, D_IN], BF16, tag="rbf", name=f"rbf{t}")
                nc.scalar.activation(rbf[:], r32[:], Act.Identity,
                                     bias=bv_pp[:, t:t + 1],
                                     scale=sv_pp[:, t:t + 1])
                for g in range(8):
                    prt = pspool.tile([P, 512], BF16, tag="bc", bufs=3,
                                      name=f"prt{t}_{g}")
                    for j in range(4):
                        kk = 4 * g + j
                        nc.tensor.transpose(
                            prt[:, j * P:(j + 1) * P],
                            rbf[:, kk * P:(kk + 1) * P], identity[:])
                    k0 = 4 * g
                    nc.vector.tensor_copy(
                        residT[:, k0:k0 + 4, t * P:(t + 1) * P], prt[:])

            def x_load(tb):
                xbf = xbfpool.tile([P, D_IN], BF16, tag="xbf",
                                   name=f"xbf{tb}")
                if tb < 2 and nbh == 1:
                    # first blocks: hwdge halves + DVE cast (SWDGE would
                    # delay the first z chains by ~20us)
                    for hf in range(2):
                        hs = slice(hf * HALF, (hf + 1) * HALF)
                        xh = rbfpool.tile([P, HALF], F32, tag="rbf",
                                          name=f"xh{tb}_{hf}")
                        nc.sync.dma_start(xh[:], x_in[tb * P:(tb + 1) * P, hs])
                        nc.vector.tensor_copy(xbf[:, hs], xh[:])
                elif tb < 6 or nbh == 2:
                    # ramp: SWDGE cast-DMA (hwdge queues are resid-busy)
                    nc.gpsimd.dma_start(xbf[:], x_in[tb * P:(tb + 1) * P, :])
                else:
                    # steady: sync hwdge is idle; f32 stage + DVE cast
                    x_f32 = r32pool.tile([P, D_IN], F32, tag="r32",
                                         name=f"xf32_{tb}")
                    nc.sync.dma_start(x_f32[:], x_in[tb * P:(tb + 1) * P, :])
                    nc.vector.tensor_copy(xbf[:], x_f32[:])
                return xbf

            def xz_block(tb, xbf):
                # one token block: xT (bf16+f8), z chain. Ramp blocks
                # transpose on the PE (sync queue is resid-busy); steady
                # blocks use the idle sync xbar and save PE cycles.
                xt = xtpool.tile([P, KC, P], BF16, tag="xt", name=f"xt{tb}")
                xf8 = xf8pool.tile([P, KC, P], F8, tag="xf8", name=f"xf8_{tb}")
                if False:
                    nc.sync.dma_start_transpose(xt[:], xbf[:])
                    nc.vector.tensor_copy(xf8[:], xt[:])
                else:
                    for g in range(8):
                        prx = pspool.tile([P, 512], BF16, tag="bc", bufs=3,
                                          name=f"prx{tb}_{g}")
                        for j in range(4):
                            kk = 4 * g + j
                            nc.tensor.transpose(prx[:, j * P:(j + 1) * P],
                                                xbf[:, kk * P:(kk + 1) * P],
                                                identity[:])
                        k0 = 4 * g
                        if g % 2 == 0:
                            nc.scalar.copy(xt[:, k0:k0 + 4, :], prx[:])
                        else:
                            nc.vector.tensor_copy(xt[:, k0:k0 + 4, :], prx[:])
                        nc.vector.tensor_copy(xf8[:, k0:k0 + 4, :], prx[:])
                # zT directly: basisT chunk stationary, xT moving
                for bh in range(nbh):
                    pz = pspool.tile([P, 512], F32, tag="z", bufs=1,
                                     name=f"pz{tb}_{bh}")
                    for k in range(KC):
                        nc.tensor.matmul(
                            pz[:, :P], lhsT=basisT[:, k, bh * P:(bh + 1) * P],
                            rhs=xt[:, k, :],
                            start=(k == 0), stop=(k == KC - 1))
                    nc.scalar.copy(zT[:, bh, tb * P:(tb + 1) * P], pz[:, :P])
                return xf8

            xf8s = [None] * NB

            def main_mm(tb, os):
                osl = slice(os * 512, (os + 1) * 512)
                tbs = slice(tb * P, (tb + 1) * P)
                xf8 = xf8s[tb]
                ps = pspool.tile([P, 512], F32, tag="mm", bufs=4,
                                 name=f"ps{tb}_{os}")
                for bh in range(nbh):
                    nc.tensor.matmul(ps[:], lhsT=zT[:, bh, tbs],
                                     rhs=G[:, bh, osl],
                                     start=(bh == 0), stop=False)
                for k2 in range(KH):
                    ks = slice(2 * k2, 2 * k2 + 2)
                    nc.tensor.matmul(ps[:], lhsT=xf8[:, ks, :],
                                     rhs=residT[:, ks, osl],
                                     start=False, stop=(k2 == KH - 1),
                                     perf_mode=DR)
                y_t = ypool.tile([P, 512], F32, tag="y", name=f"y{tb}_{os}")
                nc.vector.scalar_tensor_tensor(y_t[:], ps[:], 1.0 / CSC,
                                               bias_bc[:, osl],
                                               op0=Alu.mult, op1=Alu.add)
                nc.scalar.dma_start(y_out[tbs, osl], y_t[:])

            # ---- interleaved schedule (issue order = engine order) ---
            x_loads = [None] * NB
            x_loads[0] = x_load(0)
            x_loads[1] = x_load(1)
            for t in (0, 1, 2, 3):
                resid_step(t)
            xf8s[0] = xz_block(0, x_loads[0])
            x_loads[2] = x_load(2)
            xf8s[1] = xz_block(1, x_loads[1])
            x_loads[3] = x_load(3)
            main_mm(0, 0), main_mm(1, 0)
            for t in (4, 5):
                resid_step(t)
            xf8s[2] = xz_block(2, x_loads[2])
            x_loads[4] = x_load(4)
            xf8s[3] = xz_block(3, x_loads[3])
            x_loads[5] = x_load(5)
            main_mm(2, 0), main_mm(3, 0)
            for t in (6, 7):
                resid_step(t)
            main_mm(0, 1), main_mm(1, 1)
            for t in (8, 9):
                resid_step(t)
            main_mm(2, 1), main_mm(3, 1)
            for t in (10, 11):
                resid_step(t)
            main_mm(0, 2), main_mm(1, 2)
            for t in (12, 13):
                resid_step(t)
            main_mm(2, 2), main_mm(3, 2)
            for t in (14, 15):
                resid_step(t)
            for tb in range(4):
                main_mm(tb, 3)
            xf8s[4] = xz_block(4, x_loads[4])
            x_loads[6] = x_load(6)
            for tb in range(4, NB):
                if tb + 1 < NB:
                    xf8s[tb + 1] = xz_block(tb + 1, x_loads[tb + 1])
                    if tb + 3 < NB:
                        x_loads[tb + 3] = x_load(tb + 3)
                for os in range(NOS):
                    main_mm(tb, os)

    if split_waits:
        _split_sync_waits(nc)
    return nc


_program_cache = {}


def _get_program(nbh):
    if nbh not in _program_cache:
        _program_cache[nbh] = _build_program(nbh)
    return _program_cache[nbh]


def _find_exact_split(counts, target=O_SH):
    """128 basis values whose counts sum exactly to target, or None."""
    import random
    rng = random.Random(0)
    order = np.argsort(-counts)
    S = set(int(v) for v in order[:BASIS // 2])
    cur = int(sum(counts[v] for v in S))
    allv = list(range(BASIS))
    for _ in range(500000):
        if cur == target:
            return np.array(sorted(S), dtype=np.int64)
        a = rng.choice(tuple(S))
        b = rng.choice(allv)
        if b in S:
            continue
        delta = int(counts[b]) - int(counts[a])
        if ((cur > target and delta < 0) or (cur < target and delta > 0)
                or rng.random() < 0.02):
            S.remove(a)
            S.add(b)
            cur += delta
    return None


def _in_maps(x, codes, basis_table, residual_q, residual_scales, bias):
    """Returns (in_maps, nbh, perms). perms[oc] maps local o -> global o."""
    x2 = x.reshape(B * S, D_IN)
    idx = codes & 0xFF
    counts = np.bincount(idx, minlength=BASIS)
    S0 = _find_exact_split(counts)

    if S0 is not None:
        nbh = 1
        sel = np.isin(idx, S0)
        S1 = np.setdiff1d(np.arange(BASIS), S0)
        perms = [np.where(sel)[0], np.where(~sel)[0]]
        basis_sets = [S0, S1]
        luts = []
        for Sc in basis_sets:
            lut = np.zeros(BASIS, dtype=np.int32)
            lut[Sc] = np.arange(len(Sc), dtype=np.int32)
            luts.append(lut)
    else:
        nbh = 2
        perms = [np.arange(oc * O_SH, (oc + 1) * O_SH)
                 for oc in range(O_SHARDS)]
        basis_sets = [np.arange(BASIS), np.arange(BASIS)]
        luts = [np.arange(BASIS, dtype=np.int32)] * 2

    in_maps = []
    for core in range(N_CORES):
        oc, nb = divmod(core, N_SHARDS)
        pc = perms[oc]
        nsl = slice(nb * N_SH, (nb + 1) * N_SH)
        codes_c = codes[pc]
        # remap the idx bitfield to the local basis row numbering
        codes_c = (codes_c & ~np.int32(0xFF)) | luts[oc][codes_c & 0xFF]
        in_maps.append({
            "x_sh": np.ascontiguousarray(x2[nsl]),
            "codes_sh": np.ascontiguousarray(codes_c.astype(np.int32)),
            "basis": np.ascontiguousarray(basis_table[basis_sets[oc]]),
            "resid_sh": np.ascontiguousarray(residual_q[pc]),
            "scales_sh": np.ascontiguousarray(residual_scales[pc]),
            "bias_sh": np.ascontiguousarray(bias[pc]),
        })
    return in_maps, nbh, perms


def kernel(x, codes, basis_table, residual_q, residual_scales, bias):
    x = np.ascontiguousarray(np.asarray(x, dtype=np.float32))
    codes = np.ascontiguousarray(np.asarray(codes, dtype=np.int32))
    basis_table = np.ascontiguousarray(np.asarray(basis_table, dtype=np.float32))
    residual_q = np.ascontiguousarray(np.asarray(residual_q, dtype=np.int32))
    residual_scales = np.ascontiguousarray(
        np.asarray(residual_scales, dtype=np.float32))
    bias = np.ascontiguousarray(np.asarray(bias, dtype=np.float32))

    in_maps, nbh, perms = _in_maps(x, codes, basis_table, residual_q,
                                   residual_scales, bias)
    nc = _get_program(nbh)
    res = run_bass_kernel_spmd(nc, in_maps, core_ids=list(range(N_CORES)))

    y = np.empty((B * S, D_OUT), dtype=np.float32)
    for core in range(N_CORES):
        oc, nb = divmod(core, N_SHARDS)
        y[nb * N_SH:(nb + 1) * N_SH, perms[oc]] = res.results[core]["y_sh"]
    return y.reshape(B, S, D_OUT)


# revision 33
# speedup vs baseline: 1.0001x; 1.0001x over previous
"""BitfieldLinear (vq_codebook) Trainium2 kernel — fp8 decomposed.

y = x @ W^T + bias, W = r[:,None]*basis[idx] + s[:,None]*(q-128)/127.

Instead of materializing W in bf16 (PE floor ~437us/core), split:
  y = zT.T @ G + x_f8 @ residT_f8 * (1/C) + bias
  - z = x @ basisT in bf16; the basis term becomes a one-hot gather
    matmul with G[b,o] = C*r[o] at b=idx[o] (accurate path, small).
  - the residual matmul runs in fp8e4 DoubleRow perf mode (2x PE rate);
    residT_f8 pre-scaled by s*C/127 (C=2048 keeps values out of the
    fp8 subnormal range), x cast bf16->fp8 directly.
  Both accumulate into the same PSUM bank; evacuation is one DVE op
  (psum * 1/C + bias_bc).

Sharding across 8 NeuronCores: 2-way over out_features x 4-way over
flattened tokens (2048 tokens x 2048 outs per core, K=4096). The
out-feature split is BUCKETED by basis index: the host partitions the
256 basis rows into two sets whose assigned output counts are exactly
2048 each (dense subset-sum on the actual codes), permutes the
o-axis inputs accordingly, and un-permutes y on gather. Each core
then needs only its 128 basis rows (halves the z/base-term work).
Falls back to full-256-basis replication if no exact split exists.

All transposes run on the PE (the DMA xbar is descriptor-rate-bound);
x streams via gpsimd cast-DMA (f32->bf16); resid rows stream as
full-row i32 loads (16KB descriptors) alternating sync/scalar HWDGE
queues; decode on ACT with s*C/127 folded; PSUM evacuations on DVE.
"""

import numpy as np

import concourse.bass as bass
import concourse.mybir as mybir
import concourse.tile as tile
from concourse.masks import make_identity
from concourse.bass_utils import run_bass_kernel_spmd

# problem shape (hardcoded per harness contract)
B, S, D_IN, D_OUT, BASIS = 4, 2048, 4096, 4096, 256
N_CORES = 8
O_SHARDS, N_SHARDS = 2, 4           # grid: core = oc * N_SHARDS + nb
O_SH = D_OUT // O_SHARDS            # 2048 out-features per core
N_SH = (B * S) // N_SHARDS          # 2048 token rows per core

P = 128
KC = D_IN // P                      # 32 contraction chunks
KH = KC // 2                        # 16 chunks per D_IN half
HALF = D_IN // 2
NB = N_SH // P                      # 16 token blocks per core
NOS = O_SH // 512                   # 4 PSUM o-slices per core
OT = O_SH // P                      # 16 resid row-tiles per core
CSC = 2048.0                        # fp8 residual pre-scale

F32 = mybir.dt.float32
BF16 = mybir.dt.bfloat16
F8 = mybir.dt.float8e4
I32 = mybir.dt.int32

_WAIT_LIMIT = 1


def _split_sync_waits(nc):
    """walrus in this container rejects instructions with more than one
    embedded sync-wait command; hoist the excess onto same-engine NoOps."""
    ctr = 0
    for f in nc.m.functions:
        for bb in f.blocks:
            new = []
            changed = False
            for inst in bb.instructions:
                si = inst.sync_info
                if si is not None and si.on_wait and len(si.on_wait) > _WAIT_LIMIT:
                    waits = list(si.on_wait)
                    excess, keep = waits[:-_WAIT_LIMIT], waits[-_WAIT_LIMIT:]
                    for i in range(0, len(excess), _WAIT_LIMIT):
                        ctr += 1
                        new.append(mybir.InstNoOp(
                            name=f"I-waitsplit-{ctr}",
                            engine=inst.engine,
                            ins=[], outs=[],
                            sync_info=mybir.SyncInfo(
                                on_wait=excess[i:i + _WAIT_LIMIT], on_update=[]),
                        ))
                    si.on_wait = keep
                    changed = True
                new.append(inst)
            if changed:
                bb.instructions = new


def _build_program(nbh, split_waits=True):
    """nbh = basis halves per core: 1 (bucketed, 128 rows) or 2 (full)."""
    nc = bass.Bass()
    Alu = mybir.AluOpType
    Act = mybir.ActivationFunctionType
    DR = mybir.MatmulPerfMode.DoubleRow
    BL = nbh * P                    # local basis rows

    x_in = nc.dram_tensor("x_sh", [N_SH, D_IN], F32, kind="ExternalInput")
    codes_in = nc.dram_tensor("codes_sh", [O_SH], I32, kind="ExternalInput")
    basis_in = nc.dram_tensor("basis", [BL, D_IN], F32, kind="ExternalInput")
    resid_in = nc.dram_tensor("resid_sh", [O_SH, D_IN], I32, kind="ExternalInput")
    scales_in = nc.dram_tensor("scales_sh", [O_SH], F32, kind="ExternalInput")
    bias_in = nc.dram_tensor("bias_sh", [O_SH], F32, kind="ExternalInput")
    y_out = nc.dram_tensor("y_sh", [N_SH, O_SH], F32, kind="ExternalOutput")

    with tile.TileContext(nc) as tc:
        with (
            tc.tile_pool(name="const", bufs=1) as cpool,
            tc.tile_pool(name="rows", bufs=2) as rowpool,   # [1,512] i32
            tc.tile_pool(name="xbf", bufs=2 if nbh == 1 else 1) as xbfpool,
            tc.tile_pool(name="xt", bufs=2) as xtpool,      # [128,KC,128] bf16
            tc.tile_pool(name="xf8", bufs=5) as xf8pool,    # [128,KC,128] f8
            tc.tile_pool(name="r32", bufs=2) as r32pool,    # [128,4096] i32
            tc.tile_pool(name="rbf", bufs=2 if nbh == 1 else 1) as rbfpool,    # [128,4096] bf16
            tc.tile_pool(name="y", bufs=2) as ypool,        # [128,512] f32
            tc.tile_pool(name="psum", bufs=6, space="PSUM") as pspool,
        ):
            # ---- decode code scalars (chunked to keep scratch small) -
            idx_row_f = cpool.tile([1, O_SH], BF16, name="idx_row_f")
            r_row_f = cpool.tile([1, O_SH], BF16, name="r_row_f")
            for cq in range(4):
                cs = slice(cq * 512, (cq + 1) * 512)
                c_row = rowpool.tile([1, 512], I32, tag="row",
                                     name=f"c_row{cq}")
                nc.sync.dma_start(c_row[:], codes_in[None, cs])
                t_row = rowpool.tile([1, 512], I32, tag="row",
                                     name=f"t_row{cq}")
                nc.vector.tensor_scalar(t_row[:], c_row[:], 0xFF, None,
                                        Alu.bitwise_and)
                nc.scalar.activation(idx_row_f[:, cs], t_row[:], Act.Copy)
                nc.vector.tensor_scalar(t_row[:], c_row[:], 8, 0xFFFF,
                                        Alu.logical_shift_right,
                                        Alu.bitwise_and)
                # r scaled by C so the fp8 residual 1/C evac matches
                nc.scalar.activation(r_row_f[:, cs], t_row[:], Act.Copy,
                                     scale=CSC / 65535.0)

            # per-row decode scale/bias for the residual (ACT layout);
            # [16,128] load + PE transpose (the AP-swap DMA is 2048
            # 4-byte descriptors and stalls the sync queue at startup)
            s16 = cpool.tile([OT, P], F32, name="s16")
            nc.sync.dma_start(s16[:], scales_in.rearrange("(t p) -> t p", p=P))
            id16 = cpool.tile([OT, OT], F32, name="id16")
            make_identity(nc, id16[:])
            ps_s = pspool.tile([P, 512], F32, tag="bc", bufs=3, name="ps_s")
            nc.tensor.transpose(ps_s[:, :OT], s16[:], id16[:])
            sv_pp = cpool.tile([P, OT], F32, name="sv_pp")
            nc.vector.tensor_scalar_mul(sv_pp[:], ps_s[:, :OT], CSC / 127.0)
            bv_pp = cpool.tile([P, OT], F32, name="bv_pp")
            nc.vector.tensor_scalar_mul(bv_pp[:], ps_s[:, :OT],
                                        -128.0 * CSC / 127.0)

            bias_row = cpool.tile([1, O_SH], BF16, name="bias_row")
            nc.gpsimd.dma_start(bias_row[:], bias_in[None, :])
            ones_row = cpool.tile([1, P], BF16, name="ones_row")
            nc.vector.memset(ones_row[:], 1.0)
            identity = cpool.tile([P, P], BF16, name="identity")
            make_identity(nc, identity[:])

            # ---- one-hot G [128 b, nbh, O_SH o] and bias broadcast ---
            iota_i = cpool.tile([P, 1], I32, name="iota_i")
            nc.gpsimd.iota(iota_i[:], pattern=[[0, 1]], base=0,
                           channel_multiplier=1)
            iota_f = [cpool.tile([P, 1], F32, name=f"iota_f{bh}")
                      for bh in range(nbh)]
            nc.scalar.activation(iota_f[0][:], iota_i[:], Act.Copy)
            if nbh == 2:
                nc.scalar.activation(iota_f[1][:], iota_i[:], Act.Copy,
                                     bias=128.0, scale=1.0)
            G = cpool.tile([P, nbh, O_SH], BF16, name="G")
            bias_bc = cpool.tile([P, O_SH], BF16, name="bias_bc")
            r_bc = cpool.tile([P, 512], BF16, name="r_bc")
            for q in range(NOS):
                qs = slice(q * 512, (q + 1) * 512)
                pr = pspool.tile([P, 512], F32, tag="bc", bufs=3, name=f"pr{q}")
                nc.tensor.matmul(pr[:], lhsT=ones_row[:], rhs=r_row_f[:, qs],
                                 start=True, stop=True)
                nc.scalar.copy(r_bc[:], pr[:])
                pi = pspool.tile([P, 512], F32, tag="bc", bufs=3, name=f"pi{q}")
                nc.tensor.matmul(pi[:], lhsT=ones_row[:], rhs=idx_row_f[:, qs],
                                 start=True, stop=True)
                for bh in range(nbh):
                    nc.vector.scalar_tensor_tensor(
                        G[:, bh, qs], pi[:], iota_f[bh][:, :1], r_bc[:],
                        op0=Alu.is_equal, op1=Alu.mult)
                pb = pspool.tile([P, 512], F32, tag="bc", bufs=3, name=f"pb{q}")
                nc.tensor.matmul(pb[:], lhsT=ones_row[:], rhs=bias_row[:, qs],
                                 start=True, stop=True)
                nc.scalar.copy(bias_bc[:, qs], pb[:])

            # ---- basisT [128 i, KC k, BL b] bf16 ---------------------
            basisT = cpool.tile([P, KC, BL], BF16, name="basisT")
            for bh2 in range(nbh):
                b_f32 = r32pool.tile([P, D_IN], F32, tag="r32",
                                     name=f"bf32_{bh2}")
                nc.scalar.dma_start(b_f32[:], basis_in[bh2 * P:(bh2 + 1) * P, :])
                b_bf = xbfpool.tile([P, D_IN], BF16, tag="xbf",
                                    name=f"bbf{bh2}")
                nc.vector.tensor_copy(b_bf[:], b_f32[:])
                nc.sync.dma_start_transpose(
                    basisT[:, :, bh2 * P:(bh2 + 1) * P], b_bf[:])

            # ---- persistent W^T (residual, fp8) and zT ---------------
            residT = cpool.tile([P, KC, O_SH], F8, name="residT")
            zT = cpool.tile([P, nbh, N_SH], BF16, name="zT")

            def resid_step(t):
                # [128 o-rows, 4096 i]: full-row load (16KB descriptors),
                # ACT decode, PE transpose, DVE evac casts to f8
                r32 = r32pool.tile([P, D_IN], I32, tag="r32", name=f"r32_{t}")
                eng_a = nc.sync if t % 2 == 0 else nc.scalar
                eng_a.dma_start(r32[:], resid_in[t * P:(t + 1) * P, :])
                rbf = rbfpool.tile([P, D_IN], BF16, tag="rbf", name=f"rbf{t}")
                nc.scalar.activation(rbf[:], r32[:], Act.Identity,
                                     bias=bv_pp[:, t:t + 1],
                                     scale=sv_pp[:, t:t + 1])
                for g in range(8):
                    prt = pspool.tile([P, 512], BF16, tag="bc", bufs=3,
                                      name=f"prt{t}_{g}")
                    for j in range(4):
                        kk = 4 * g + j
                        nc.tensor.transpose(
                            prt[:, j * P:(j + 1) * P],
                            rbf[:, kk * P:(kk + 1) * P], identity[:])
                    k0 = 4 * g
                    nc.vector.tensor_copy(
                        residT[:, k0:k0 + 4, t * P:(t + 1) * P], prt[:])

            def x_load(tb):
                xbf = xbfpool.tile([P, D_IN], BF16, tag="xbf",
                                   name=f"xbf{tb}")
                if tb < 2 and nbh == 1:
                    # first blocks: hwdge halves + DVE cast (SWDGE would
                    # delay the first z chains by ~20us)
                    for hf in range(2):
                        hs = slice(hf * HALF, (hf + 1) * HALF)
                        xh = rbfpool.tile([P, HALF], F32, tag="rbf",
                                          name=f"xh{tb}_{hf}")
                        nc.sync.dma_start(xh[:], x_in[tb * P:(tb + 1) * P, hs])
                        nc.vector.tensor_copy(xbf[:, hs], xh[:])
                elif tb < 6 or nbh == 2:
                    # ramp: SWDGE cast-DMA (hwdge queues are resid-busy)
                    nc.gpsimd.dma_start(xbf[:], x_in[tb * P:(tb + 1) * P, :])
                else:
                    # steady: sync hwdge is idle; f32 stage + DVE cast
                    x_f32 = r32pool.tile([P, D_IN], F32, tag="r32",
                                         name=f"xf32_{tb}")
                    nc.sync.dma_start(x_f32[:], x_in[tb * P:(tb + 1) * P, :])
                    nc.vector.tensor_copy(xbf[:], x_f32[:])
                return xbf

            def xz_block(tb, xbf):
                # one token block: xT (bf16+f8), z chain. Ramp blocks
                # transpose on the PE (sync queue is resid-busy); steady
                # blocks use the idle sync xbar and save PE cycles.
                xt = xtpool.tile([P, KC, P], BF16, tag="xt", name=f"xt{tb}")
                xf8 = xf8pool.tile([P, KC, P], F8, tag="xf8", name=f"xf8_{tb}")
                if False:
                    nc.sync.dma_start_transpose(xt[:], xbf[:])
                    nc.vector.tensor_copy(xf8[:], xt[:])
                else:
                    for g in range(8):
                        prx = pspool.tile([P, 512], BF16, tag="bc", bufs=3,
                                          name=f"prx{tb}_{g}")
                        for j in range(4):
                            kk = 4 * g + j
                            nc.tensor.transpose(prx[:, j * P:(j + 1) * P],
                                                xbf[:, kk * P:(kk + 1) * P],
                                                identity[:])
                        k0 = 4 * g
                        if g % 2 == 0:
                            nc.scalar.copy(xt[:, k0:k0 + 4, :], prx[:])
                        else:
                            nc.vector.tensor_copy(xt[:, k0:k0 + 4, :], prx[:])
                        nc.vector.tensor_copy(xf8[:, k0:k0 + 4, :], prx[:])
                # zT directly: basisT chunk stationary, xT moving
                for bh in range(nbh):
                    pz = pspool.tile([P, 512], F32, tag="z", bufs=1,
                                     name=f"pz{tb}_{bh}")
                    for k in range(KC):
                        nc.tensor.matmul(
                            pz[:, :P], lhsT=basisT[:, k, bh * P:(bh + 1) * P],
                            rhs=xt[:, k, :],
                            start=(k == 0), stop=(k == KC - 1))
                    nc.scalar.copy(zT[:, bh, tb * P:(tb + 1) * P], pz[:, :P])
                return xf8

            xf8s = [None] * NB

            def main_mm(tb, os):
                osl = slice(os * 512, (os + 1) * 512)
                tbs = slice(tb * P, (tb + 1) * P)
                xf8 = xf8s[tb]
                ps = pspool.tile([P, 512], F32, tag="mm", bufs=4,
                                 name=f"ps{tb}_{os}")
                for bh in range(nbh):
                    nc.tensor.matmul(ps[:], lhsT=zT[:, bh, tbs],
                                     rhs=G[:, bh, osl],
                                     start=(bh == 0), stop=False)
                for k2 in range(KH):
                    ks = slice(2 * k2, 2 * k2 + 2)
                    nc.tensor.matmul(ps[:], lhsT=xf8[:, ks, :],
                                     rhs=residT[:, ks, osl],
                                     start=False, stop=(k2 == KH - 1),
                                     perf_mode=DR)
                y_t = ypool.tile([P, 512], F32, tag="y", name=f"y{tb}_{os}")
                nc.vector.scalar_tensor_tensor(y_t[:], ps[:], 1.0 / CSC,
                                               bias_bc[:, osl],
                                               op0=Alu.mult, op1=Alu.add)
                nc.scalar.dma_start(y_out[tbs, osl], y_t[:])

            # ---- interleaved schedule (issue order = engine order) ---
            x_loads = [None] * NB
            x_loads[0] = x_load(0)
            x_loads[1] = x_load(1)
            for t in (0, 1, 2, 3):
                resid_step(t)
            xf8s[0] = xz_block(0, x_loads[0])
            x_loads[2] = x_load(2)
            xf8s[1] = xz_block(1, x_loads[1])
            x_loads[3] = x_load(3)
            main_mm(0, 0), main_mm(1, 0)
            for t in (4, 5):
                resid_step(t)
            xf8s[2] = xz_block(2, x_loads[2])
            x_loads[4] = x_load(4)
            xf8s[3] = xz_block(3, x_loads[3])
            x_loads[5] = x_load(5)
            main_mm(2, 0), main_mm(3, 0)
            for t in (6, 7):
                resid_step(t)
            main_mm(0, 1), main_mm(1, 1)
            for t in (8, 9):
                resid_step(t)
            main_mm(2, 1), main_mm(3, 1)
            xf8s[4] = xz_block(4, x_loads[4])
            x_loads[6] = x_load(6)
            for t in (10, 11):
                resid_step(t)
            main_mm(0, 2), main_mm(1, 2)
            for t in (12, 13):
                resid_step(t)
            main_mm(2, 2), main_mm(3, 2)
            for t in (14, 15):
                resid_step(t)
            for tb in range(4):
                main_mm(tb, 3)
            for tb in range(4, NB):
                if tb + 1 < NB:
                    xf8s[tb + 1] = xz_block(tb + 1, x_loads[tb + 1])
                    if tb + 3 < NB:
                        x_loads[tb + 3] = x_load(tb + 3)
                for os in range(NOS):
                    main_mm(tb, os)

    if split_waits:
        _split_sync_waits(nc)
    return nc


_program_cache = {}


def _get_program(nbh):
    if nbh not in _program_cache:
        _program_cache[nbh] = _build_program(nbh)
    return _program_cache[nbh]


def _find_exact_split(counts, target=O_SH):
    """128 basis values whose counts sum exactly to target, or None."""
    import random
    rng = random.Random(0)
    order = np.argsort(-counts)
    S = set(int(v) for v in order[:BASIS // 2])
    cur = int(sum(counts[v] for v in S))
    allv = list(range(BASIS))
    for _ in range(500000):
        if cur == target:
            return np.array(sorted(S), dtype=np.int64)
        a = rng.choice(tuple(S))
        b = rng.choice(allv)
        if b in S:
            continue
        delta = int(counts[b]) - int(counts[a])
        if ((cur > target and delta < 0) or (cur < target and delta > 0)
                or rng.random() < 0.02):
            S.remove(a)
            S.add(b)
            cur += delta
    return None


def _in_maps(x, codes, basis_table, residual_q, residual_scales, bias):
    """Returns (in_maps, nbh, perms). perms[oc] maps local o -> global o."""
    x2 = x.reshape(B * S, D_IN)
    idx = codes & 0xFF
    counts = np.bincount(idx, minlength=BASIS)
    S0 = _find_exact_split(counts)

    if S0 is not None:
        nbh = 1
        sel = np.isin(idx, S0)
        S1 = np.setdiff1d(np.arange(BASIS), S0)
        perms = [np.where(sel)[0], np.where(~sel)[0]]
        basis_sets = [S0, S1]
        luts = []
        for Sc in basis_sets:
            lut = np.zeros(BASIS, dtype=np.int32)
            lut[Sc] = np.arange(len(Sc), dtype=np.int32)
            luts.append(lut)
    else:
        nbh = 2
        perms = [np.arange(oc * O_SH, (oc + 1) * O_SH)
                 for oc in range(O_SHARDS)]
        basis_sets = [np.arange(BASIS), np.arange(BASIS)]
        luts = [np.arange(BASIS, dtype=np.int32)] * 2

    in_maps = []
    for core in range(N_CORES):
        oc, nb = divmod(core, N_SHARDS)
        pc = perms[oc]
        nsl = slice(nb * N_SH, (nb + 1) * N_SH)
        codes_c = codes[pc]
        # remap the idx bitfield to the local basis row numbering
        codes_c = (codes_c & ~np.int32(0xFF)) | luts[oc][codes_c & 0xFF]
        in_maps.append({
            "x_sh": np.ascontiguousarray(x2[nsl]),
            "codes_sh": np.ascontiguousarray(codes_c.astype(np.int32)),
            "basis": np.ascontiguousarray(basis_table[basis_sets[oc]]),
            "resid_sh": np.ascontiguousarray(residual_q[pc]),
            "scales_sh": np.ascontiguousarray(residual_scales[pc]),
            "bias_sh": np.ascontiguousarray(bias[pc]),
        })
    return in_maps, nbh, perms


def kernel(x, codes, basis_table, residual_q, residual_scales, bias):
    x = np.ascontiguousarray(np.asarray(x, dtype=np.float32))
    codes = np.ascontiguousarray(np.asarray(codes, dtype=np.int32))
    basis_table = np.ascontiguousarray(np.asarray(basis_table, dtype=np.float32))
    residual_q = np.ascontiguousarray(np.asarray(residual_q, dtype=np.int32))
    residual_scales = np.ascontiguousarray(
        np.asarray(residual_scales, dtype=np.float32))
    bias = np.ascontiguousarray(np.asarray(bias, dtype=np.float32))

    in_maps, nbh, perms = _in_maps(x, codes, basis_table, residual_q,
                                   residual_scales, bias)
    nc = _get_program(nbh)
    res = run_bass_kernel_spmd(nc, in_maps, core_ids=list(range(N_CORES)))

    y = np.empty((B * S, D_OUT), dtype=np.float32)
    for core in range(N_CORES):
        oc, nb = divmod(core, N_SHARDS)
        y[nb * N_SH:(nb + 1) * N_SH, perms[oc]] = res.results[core]["y_sh"]
    return y.reshape(B, S, D_OUT)


# revision 35
# speedup vs baseline: 1.0235x; 1.0234x over previous
"""BitfieldLinear (vq_codebook) Trainium2 kernel — fp8 decomposed.

y = x @ W^T + bias, W = r[:,None]*basis[idx] + s[:,None]*(q-128)/127.

Instead of materializing W in bf16 (PE floor ~437us/core), split:
  y = zT.T @ G + x_f8 @ residT_f8 * (1/C) + bias
  - z = x @ basisT in bf16; the basis term becomes a one-hot gather
    matmul with G[b,o] = C*r[o] at b=idx[o] (accurate path, small).
  - the residual matmul runs in fp8e4 DoubleRow perf mode (2x PE rate);
    residT_f8 pre-scaled by s*C/127 (C=2048 keeps values out of the
    fp8 subnormal range), x cast bf16->fp8 directly.
  Both accumulate into the same PSUM bank; evacuation is one DVE op
  (psum * 1/C + bias_bc).

Sharding across 8 NeuronCores: 2-way over out_features x 4-way over
flattened tokens (2048 tokens x 2048 outs per core, K=4096). The
out-feature split is BUCKETED by basis index: the host partitions the
256 basis rows into two sets whose assigned output counts are exactly
2048 each (dense subset-sum on the actual codes), permutes the
o-axis inputs accordingly, and un-permutes y on gather. Each core
then needs only its 128 basis rows (halves the z/base-term work).
Falls back to full-256-basis replication if no exact split exists.

All transposes run on the PE (the DMA xbar is descriptor-rate-bound);
x streams via gpsimd cast-DMA (f32->bf16); resid rows stream as
full-row i32 loads (16KB descriptors) alternating sync/scalar HWDGE
queues; decode on ACT with s*C/127 folded; PSUM evacuations on DVE.
"""

import numpy as np

import concourse.bass as bass
import concourse.mybir as mybir
import concourse.tile as tile
from concourse.masks import make_identity
from concourse.bass_utils import run_bass_kernel_spmd

# problem shape (hardcoded per harness contract)
B, S, D_IN, D_OUT, BASIS = 4, 2048, 4096, 4096, 256
N_CORES = 8
O_SHARDS, N_SHARDS = 2, 4           # grid: core = oc * N_SHARDS + nb
O_SH = D_OUT // O_SHARDS            # 2048 out-features per core
N_SH = (B * S) // N_SHARDS          # 2048 token rows per core

P = 128
KC = D_IN // P                      # 32 contraction chunks
KH = KC // 2                        # 16 chunks per D_IN half
HALF = D_IN // 2
NB = N_SH // P                      # 16 token blocks per core
NOS = O_SH // 512                   # 4 PSUM o-slices per core
OT = O_SH // P                      # 16 resid row-tiles per core
CSC = 2048.0                        # fp8 residual pre-scale

F32 = mybir.dt.float32
BF16 = mybir.dt.bfloat16
F8 = mybir.dt.float8e4
I32 = mybir.dt.int32

_WAIT_LIMIT = 1


def _split_sync_waits(nc):
    """walrus in this container rejects instructions with more than one
    embedded sync-wait command; hoist the excess onto same-engine NoOps."""
    ctr = 0
    for f in nc.m.functions:
        for bb in f.blocks:
            new = []
            changed = False
            for inst in bb.instructions:
                si = inst.sync_info
                if si is not None and si.on_wait and len(si.on_wait) > _WAIT_LIMIT:
                    waits = list(si.on_wait)
                    excess, keep = waits[:-_WAIT_LIMIT], waits[-_WAIT_LIMIT:]
                    for i in range(0, len(excess), _WAIT_LIMIT):
                        ctr += 1
                        new.append(mybir.InstNoOp(
                            name=f"I-waitsplit-{ctr}",
                            engine=inst.engine,
                            ins=[], outs=[],
                            sync_info=mybir.SyncInfo(
                                on_wait=excess[i:i + _WAIT_LIMIT], on_update=[]),
                        ))
                    si.on_wait = keep
                    changed = True
                new.append(inst)
            if changed:
                bb.instructions = new


def _build_program(nbh, split_waits=True):
    """nbh = basis halves per core: 1 (bucketed, 128 rows) or 2 (full)."""
    nc = bass.Bass()
    Alu = mybir.AluOpType
    Act = mybir.ActivationFunctionType
    DR = mybir.MatmulPerfMode.DoubleRow
    BL = nbh * P                    # local basis rows

    x_in = nc.dram_tensor("x_sh", [N_SH, D_IN], F32, kind="ExternalInput")
    codes_in = nc.dram_tensor("codes_sh", [O_SH], I32, kind="ExternalInput")
    basis_in = nc.dram_tensor("basis", [BL, D_IN], F32, kind="ExternalInput")
    resid_in = nc.dram_tensor("resid_sh", [O_SH, D_IN], I32, kind="ExternalInput")
    scales_in = nc.dram_tensor("scales_sh", [O_SH], F32, kind="ExternalInput")
    bias_in = nc.dram_tensor("bias_sh", [O_SH], F32, kind="ExternalInput")
    y_out = nc.dram_tensor("y_sh", [N_SH, O_SH], F32, kind="ExternalOutput")

    with tile.TileContext(nc) as tc:
        with (
            tc.tile_pool(name="const", bufs=1) as cpool,
            tc.tile_pool(name="rows", bufs=2) as rowpool,   # [1,512] i32
            tc.tile_pool(name="xbf", bufs=2 if nbh == 1 else 1) as xbfpool,
            tc.tile_pool(name="xt", bufs=2) as xtpool,      # [128,KC,128] bf16
            tc.tile_pool(name="xf8", bufs=5) as xf8pool,    # [128,KC,128] f8
            tc.tile_pool(name="r32", bufs=2) as r32pool,    # [128,4096] i32
            tc.tile_pool(name="rbf", bufs=2 if nbh == 1 else 1) as rbfpool,    # [128,4096] bf16
            tc.tile_pool(name="y", bufs=2) as ypool,        # [128,512] f32
            tc.tile_pool(name="psum", bufs=6, space="PSUM") as pspool,
        ):
            # ---- decode code scalars (chunked to keep scratch small) -
            idx_row_f = cpool.tile([1, O_SH], BF16, name="idx_row_f")
            r_row_f = cpool.tile([1, O_SH], BF16, name="r_row_f")
            for cq in range(4):
                cs = slice(cq * 512, (cq + 1) * 512)
                c_row = rowpool.tile([1, 512], I32, tag="row",
                                     name=f"c_row{cq}")
                nc.sync.dma_start(c_row[:], codes_in[None, cs])
                t_row = rowpool.tile([1, 512], I32, tag="row",
                                     name=f"t_row{cq}")
                nc.vector.tensor_scalar(t_row[:], c_row[:], 0xFF, None,
                                        Alu.bitwise_and)
                nc.scalar.activation(idx_row_f[:, cs], t_row[:], Act.Copy)
                nc.vector.tensor_scalar(t_row[:], c_row[:], 8, 0xFFFF,
                                        Alu.logical_shift_right,
                                        Alu.bitwise_and)
                # r scaled by C so the fp8 residual 1/C evac matches
                nc.scalar.activation(r_row_f[:, cs], t_row[:], Act.Copy,
                                     scale=CSC / 65535.0)

            # per-row decode scale/bias for the residual (ACT layout);
            # [16,128] load + PE transpose (the AP-swap DMA is 2048
            # 4-byte descriptors and stalls the sync queue at startup)
            s16 = cpool.tile([OT, P], F32, name="s16")
            nc.sync.dma_start(s16[:], scales_in.rearrange("(t p) -> t p", p=P))
            id16 = cpool.tile([OT, OT], F32, name="id16")
            make_identity(nc, id16[:])
            ps_s = pspool.tile([P, 512], F32, tag="bc", bufs=3, name="ps_s")
            nc.tensor.transpose(ps_s[:, :OT], s16[:], id16[:])
            sv_pp = cpool.tile([P, OT], F32, name="sv_pp")
            nc.vector.tensor_scalar_mul(sv_pp[:], ps_s[:, :OT], CSC / 127.0)
            bv_pp = cpool.tile([P, OT], F32, name="bv_pp")
            nc.vector.tensor_scalar_mul(bv_pp[:], ps_s[:, :OT],
                                        -128.0 * CSC / 127.0)

            bias_row = cpool.tile([1, O_SH], BF16, name="bias_row")
            nc.gpsimd.dma_start(bias_row[:], bias_in[None, :])
            ones_row = cpool.tile([1, P], BF16, name="ones_row")
            nc.vector.memset(ones_row[:], 1.0)
            identity = cpool.tile([P, P], BF16, name="identity")
            make_identity(nc, identity[:])

            # ---- one-hot G [128 b, nbh, O_SH o] and bias broadcast ---
            iota_i = cpool.tile([P, 1], I32, name="iota_i")
            nc.gpsimd.iota(iota_i[:], pattern=[[0, 1]], base=0,
                           channel_multiplier=1)
            iota_f = [cpool.tile([P, 1], F32, name=f"iota_f{bh}")
                      for bh in range(nbh)]
            nc.scalar.activation(iota_f[0][:], iota_i[:], Act.Copy)
            if nbh == 2:
                nc.scalar.activation(iota_f[1][:], iota_i[:], Act.Copy,
                                     bias=128.0, scale=1.0)
            G = cpool.tile([P, nbh, O_SH], BF16, name="G")
            bias_bc = cpool.tile([P, O_SH], BF16, name="bias_bc")
            r_bc = cpool.tile([P, 512], BF16, name="r_bc")
            for q in range(NOS):
                qs = slice(q * 512, (q + 1) * 512)
                pr = pspool.tile([P, 512], F32, tag="bc", bufs=3, name=f"pr{q}")
                nc.tensor.matmul(pr[:], lhsT=ones_row[:], rhs=r_row_f[:, qs],
                                 start=True, stop=True)
                nc.scalar.copy(r_bc[:], pr[:])
                pi = pspool.tile([P, 512], F32, tag="bc", bufs=3, name=f"pi{q}")
                nc.tensor.matmul(pi[:], lhsT=ones_row[:], rhs=idx_row_f[:, qs],
                                 start=True, stop=True)
                for bh in range(nbh):
                    nc.vector.scalar_tensor_tensor(
                        G[:, bh, qs], pi[:], iota_f[bh][:, :1], r_bc[:],
                        op0=Alu.is_equal, op1=Alu.mult)
                pb = pspool.tile([P, 512], F32, tag="bc", bufs=3, name=f"pb{q}")
                nc.tensor.matmul(pb[:], lhsT=ones_row[:], rhs=bias_row[:, qs],
                                 start=True, stop=True)
                nc.scalar.copy(bias_bc[:, qs], pb[:])

            # ---- basisT [128 i, KC k, BL b] bf16 ---------------------
            basisT = cpool.tile([P, KC, BL], BF16, name="basisT")
            for bh2 in range(nbh):
                b_f32 = r32pool.tile([P, D_IN], F32, tag="r32",
                                     name=f"bf32_{bh2}")
                nc.scalar.dma_start(b_f32[:], basis_in[bh2 * P:(bh2 + 1) * P, :])
                b_bf = xbfpool.tile([P, D_IN], BF16, tag="xbf",
                                    name=f"bbf{bh2}")
                nc.vector.tensor_copy(b_bf[:], b_f32[:])
                nc.sync.dma_start_transpose(
                    basisT[:, :, bh2 * P:(bh2 + 1) * P], b_bf[:])

            # ---- persistent W^T (residual, fp8) and zT ---------------
            residT = cpool.tile([P, KC, O_SH], F8, name="residT")
            zT = cpool.tile([P, nbh, N_SH], BF16, name="zT")

            def resid_step(t):
                # [128 o-rows, 4096 i]: full-row load (16KB descriptors),
                # ACT decode, PE transpose, DVE evac casts to f8
                r32 = r32pool.tile([P, D_IN], I32, tag="r32", name=f"r32_{t}")
                eng_a = nc.sync if t % 2 == 0 else nc.scalar
                eng_a.dma_start(r32[:], resid_in[t * P:(t + 1) * P, :])
                rbf = rbfpool.tile([P, D_IN], BF16, tag="rbf", name=f"rbf{t}")
                nc.scalar.activation(rbf[:], r32[:], Act.Identity,
                                     bias=bv_pp[:, t:t + 1],
                                     scale=sv_pp[:, t:t + 1])
                for g in range(8):
                    prt = pspool.tile([P, 512], BF16, tag="bc", bufs=3,
                                      name=f"prt{t}_{g}")
                    for j in range(4):
                        kk = 4 * g + j
                        nc.tensor.transpose(
                            prt[:, j * P:(j + 1) * P],
                            rbf[:, kk * P:(kk + 1) * P], identity[:])
                    k0 = 4 * g
                    nc.vector.tensor_copy(
                        residT[:, k0:k0 + 4, t * P:(t + 1) * P], prt[:])

            def x_load(tb):
                xbf = xbfpool.tile([P, D_IN], BF16, tag="xbf",
                                   name=f"xbf{tb}")
                if tb < 2 and nbh == 1:
                    # first blocks: hwdge halves + DVE cast (SWDGE would
                    # delay the first z chains by ~20us)
                    for hf in range(2):
                        hs = slice(hf * HALF, (hf + 1) * HALF)
                        xh = rbfpool.tile([P, HALF], F32, tag="rbf",
                                          name=f"xh{tb}_{hf}")
                        nc.sync.dma_start(xh[:], x_in[tb * P:(tb + 1) * P, hs])
                        nc.vector.tensor_copy(xbf[:, hs], xh[:])
                elif tb < 6 or nbh == 2:
                    # ramp: SWDGE cast-DMA (hwdge queues are resid-busy)
                    nc.gpsimd.dma_start(xbf[:], x_in[tb * P:(tb + 1) * P, :])
                else:
                    # steady: sync hwdge is idle; f32 stage + DVE cast
                    x_f32 = r32pool.tile([P, D_IN], F32, tag="r32",
                                         name=f"xf32_{tb}")
                    nc.sync.dma_start(x_f32[:], x_in[tb * P:(tb + 1) * P, :])
                    nc.vector.tensor_copy(xbf[:], x_f32[:])
                return xbf

            def xz_block(tb, xbf):
                # one token block: xT (bf16+f8) via PE transpose, z chain
                xt = xtpool.tile([P, KC, P], BF16, tag="xt", name=f"xt{tb}")
                xf8 = xf8pool.tile([P, KC, P], F8, tag="xf8", name=f"xf8_{tb}")
                for g in range(8):
                    prx = pspool.tile([P, 512], BF16, tag="bc", bufs=3,
                                      name=f"prx{tb}_{g}")
                    for j in range(4):
                        kk = 4 * g + j
                        nc.tensor.transpose(prx[:, j * P:(j + 1) * P],
                                            xbf[:, kk * P:(kk + 1) * P],
                                            identity[:])
                    k0 = 4 * g
                    if g % 2 == 0:
                        nc.scalar.copy(xt[:, k0:k0 + 4, :], prx[:])
                    else:
                        nc.vector.tensor_copy(xt[:, k0:k0 + 4, :], prx[:])
                    nc.vector.tensor_copy(xf8[:, k0:k0 + 4, :], prx[:])
                # zT directly: basisT chunk stationary, xT moving
                for bh in range(nbh):
                    pz = pspool.tile([P, 512], F32, tag="z", bufs=1,
                                     name=f"pz{tb}_{bh}")
                    for k in range(KC):
                        nc.tensor.matmul(
                            pz[:, :P], lhsT=basisT[:, k, bh * P:(bh + 1) * P],
                            rhs=xt[:, k, :],
                            start=(k == 0), stop=(k == KC - 1))
                    nc.scalar.copy(zT[:, bh, tb * P:(tb + 1) * P], pz[:, :P])
                return xf8

            xf8s = [None] * NB

            def main_mm(tb, os):
                osl = slice(os * 512, (os + 1) * 512)
                tbs = slice(tb * P, (tb + 1) * P)
                xf8 = xf8s[tb]
                ps = pspool.tile([P, 512], F32, tag="mm", bufs=4,
                                 name=f"ps{tb}_{os}")
                for bh in range(nbh):
                    nc.tensor.matmul(ps[:], lhsT=zT[:, bh, tbs],
                                     rhs=G[:, bh, osl],
                                     start=(bh == 0), stop=False)
                for k2 in range(KH):
                    ks = slice(2 * k2, 2 * k2 + 2)
                    nc.tensor.matmul(ps[:], lhsT=xf8[:, ks, :],
                                     rhs=residT[:, ks, osl],
                                     start=False, stop=(k2 == KH - 1),
                                     perf_mode=DR)
                y_t = ypool.tile([P, 512], F32, tag="y", name=f"y{tb}_{os}")
                nc.vector.scalar_tensor_tensor(y_t[:], ps[:], 1.0 / CSC,
                                               bias_bc[:, osl],
                                               op0=Alu.mult, op1=Alu.add)
                nc.scalar.dma_start(y_out[tbs, osl], y_t[:])

            # ---- interleaved schedule (issue order = engine order) ---
            x_loads = [None] * NB
            x_loads[0] = x_load(0)
            x_loads[1] = x_load(1)
            for t in (0, 1, 2, 3):
                resid_step(t)
            xf8s[0] = xz_block(0, x_loads[0])
            x_loads[2] = x_load(2)
            xf8s[1] = xz_block(1, x_loads[1])
            x_loads[3] = x_load(3)
            main_mm(0, 0), main_mm(1, 0)
            for t in (4, 5):
                resid_step(t)
            xf8s[2] = xz_block(2, x_loads[2])
            x_loads[4] = x_load(4)
            xf8s[3] = xz_block(3, x_loads[3])
            x_loads[5] = x_load(5)
            main_mm(2, 0), main_mm(3, 0)
            for t in (6, 7):
                resid_step(t)
            main_mm(0, 1), main_mm(1, 1)
            for t in (8, 9):
                resid_step(t)
            main_mm(2, 1), main_mm(3, 1)
            xf8s[4] = xz_block(4, x_loads[4])
            x_loads[6] = x_load(6)
            for t in (10, 11):
                resid_step(t)
            main_mm(0, 2), main_mm(1, 2)
            for t in (12, 13):
                resid_step(t)
            main_mm(2, 2), main_mm(3, 2)
            for t in (14, 15):
                resid_step(t)
            for tb in range(4):
                main_mm(tb, 3)
            for tb in range(4, NB):
                if tb + 1 < NB:
                    xf8s[tb + 1] = xz_block(tb + 1, x_loads[tb + 1])
                    if tb + 3 < NB:
                        x_loads[tb + 3] = x_load(tb + 3)
                for os in range(NOS):
                    main_mm(tb, os)

    if split_waits:
        _split_sync_waits(nc)
    return nc


_program_cache = {}


def _get_program(nbh):
    if nbh not in _program_cache:
        _program_cache[nbh] = _build_program(nbh)
    return _program_cache[nbh]


def _find_exact_split(counts, target=O_SH):
    """128 basis values whose counts sum exactly to target, or None."""
    import random
    rng = random.Random(0)
    order = np.argsort(-counts)
    S = set(int(v) for v in order[:BASIS // 2])
    cur = int(sum(counts[v] for v in S))
    allv = list(range(BASIS))
    for _ in range(500000):
        if cur == target:
            return np.array(sorted(S), dtype=np.int64)
        a = rng.choice(tuple(S))
        b = rng.choice(allv)
        if b in S:
            continue
        delta = int(counts[b]) - int(counts[a])
        if ((cur > target and delta < 0) or (cur < target and delta > 0)
                or rng.random() < 0.02):
            S.remove(a)
            S.add(b)
            cur += delta
    return None


def _in_maps(x, codes, basis_table, residual_q, residual_scales, bias):
    """Returns (in_maps, nbh, perms). perms[oc] maps local o -> global o."""
    x2 = x.reshape(B * S, D_IN)
    idx = codes & 0xFF
    counts = np.bincount(idx, minlength=BASIS)
    S0 = _find_exact_split(counts)

    if S0 is not None:
        nbh = 1
        sel = np.isin(idx, S0)
        S1 = np.setdiff1d(np.arange(BASIS), S0)
        perms = [np.where(sel)[0], np.where(~sel)[0]]
        basis_sets = [S0, S1]
        luts = []
        for Sc in basis_sets:
            lut = np.zeros(BASIS, dtype=np.int32)
            lut[Sc] = np.arange(len(Sc), dtype=np.int32)
            luts.append(lut)
    else:
        nbh = 2
        perms = [np.arange(oc * O_SH, (oc + 1) * O_SH)
                 for oc in range(O_SHARDS)]
        basis_sets = [np.arange(BASIS), np.arange(BASIS)]
        luts = [np.arange(BASIS, dtype=np.int32)] * 2

    in_maps = []
    for core in range(N_CORES):
        oc, nb = divmod(core, N_SHARDS)
        pc = perms[oc]
        nsl = slice(nb * N_SH, (nb + 1) * N_SH)
        codes_c = codes[pc]
        # remap the idx bitfield to the local basis row numbering
        codes_c = (codes_c & ~np.int32(0xFF)) | luts[oc][codes_c & 0xFF]
        in_maps.append({
            "x_sh": np.ascontiguousarray(x2[nsl]),
            "codes_sh": np.ascontiguousarray(codes_c.astype(np.int32)),
            "basis": np.ascontiguousarray(basis_table[basis_sets[oc]]),
            "resid_sh": np.ascontiguousarray(residual_q[pc]),
            "scales_sh": np.ascontiguousarray(residual_scales[pc]),
            "bias_sh": np.ascontiguousarray(bias[pc]),
        })
    return in_maps, nbh, perms


def kernel(x, codes, basis_table, residual_q, residual_scales, bias):
    x = np.ascontiguousarray(np.asarray(x, dtype=np.float32))
    codes = np.ascontiguousarray(np.asarray(codes, dtype=np.int32))
    basis_table = np.ascontiguousarray(np.asarray(basis_table, dtype=np.float32))
    residual_q = np.ascontiguousarray(np.asarray(residual_q, dtype=np.int32))
    residual_scales = np.ascontiguousarray(
        np.asarray(residual_scales, dtype=np.float32))
    bias = np.ascontiguousarray(np.asarray(bias, dtype=np.float32))

    in_maps, nbh, perms = _in_maps(x, codes, basis_table, residual_q,
                                   residual_scales, bias)
    nc = _get_program(nbh)
    res = run_bass_kernel_spmd(nc, in_maps, core_ids=list(range(N_CORES)))

    y = np.empty((B * S, D_OUT), dtype=np.float32)
    for core in range(N_CORES):
        oc, nb = divmod(core, N_SHARDS)
        y[nb * N_SH:(nb + 1) * N_SH, perms[oc]] = res.results[core]["y_sh"]
    return y.reshape(B, S, D_OUT)
